# revision 8
# baseline (speedup 1.0000x reference)
"""ALayer kernel for 8 TRN2 NeuronCores — pure data parallel over batch.

Per-core shard: 4 images of [256, 56, 56].
  h  = relu(conv3x3(x_in, w1))      # 256 -> 16 ch
  A  = sigmoid(conv3x3(h, w2))      # 16 -> 1 ch
  out = x_out * box3x3(A)           # broadcast over 256 ch

Formulation on the TensorEngine (all matmuls bf16, fp32 PSUM accum):
  conv1: 18 accumulating shift-matmuls (2 K-chunks of 128 in-ch x 9 taps,
         M=16 out-ch) over zero-padded 58x58 planes, 8 output rows per tile.
  conv2: relu writes h to 3 dx-shifted partition groups (H3, K=48), so
         conv2 is 3 accumulating matmuls (one per dy).
  box+broadcast: sigmoid output A is stored at 3 dx-shifted partitions (A3);
         3 accumulating matmuls with lhsT=ones[3,128] produce
         box3x3(A) replicated to 128 partitions directly in PSUM.
  final: DVE multiply x_out tile by the broadcast PSUM tile.
"""

import numpy as np
import ml_dtypes

import concourse.bass as bass
import concourse.tile as tile
import concourse.mybir as mybir
from concourse import bacc
from concourse.bass_utils import run_bass_kernel_spmd

BF16 = mybir.dt.bfloat16
F32 = mybir.dt.float32

B, C, H, W = 32, 256, 56, 56
NCORES = 8
BL = B // NCORES          # images per core
KCH = 2                   # 256 = 2 chunks of 128
HP = H + 2                # padded plane side
HW = H * W                # 3136
RG = 7                    # row groups per image
RROWS = H // RG           # 8 rows per group
NT = RROWS * W            # 448 px per tile

_cache = {}


def _build():
    nc = bacc.Bacc("TRN2", target_bir_lowering=False, debug=False)

    xin_d = nc.dram_tensor("xin", [BL, KCH, 128, HW], F32, kind="ExternalInput").ap()
    xout_d = nc.dram_tensor("xout", [BL, KCH, 128, HW], F32, kind="ExternalInput").ap()
    w1_d = nc.dram_tensor("w1t", [KCH, 9, 128, 16], BF16, kind="ExternalInput").ap()
    w2_d = nc.dram_tensor("w2t", [96, 3], BF16, kind="ExternalInput").ap()
    out_d = nc.dram_tensor("out", [BL, KCH, 128, HW], F32, kind="ExternalOutput").ap()

    with tile.TileContext(nc) as tc:
        with (
            tc.tile_pool(name="const", bufs=1) as constp,
            tc.tile_pool(name="xstage", bufs=4) as xstage,
            tc.tile_pool(name="xpad", bufs=2) as xpadp,
            tc.tile_pool(name="h3", bufs=2) as h3p,
            tc.tile_pool(name="a3", bufs=2) as a3p,
            tc.tile_pool(name="xo", bufs=4) as xop,
            tc.tile_pool(name="ot", bufs=4) as otp,
            tc.tile_pool(name="ps_h", bufs=2, space="PSUM") as ps_h,
            tc.tile_pool(name="ps_a", bufs=2, space="PSUM") as ps_a,
            tc.tile_pool(name="ps_b", bufs=2, space="PSUM") as ps_b,
        ):
            # weights
            w1sb = constp.tile([128, KCH, 9, 16], BF16)
            nc.sync.dma_start(w1sb[:], w1_d.transpose([2, 0, 1, 3]))
            w2sb = constp.tile([96, 3], BF16)
            nc.sync.dma_start(w2sb[:], w2_d[:])
            # lhsT for fused box+broadcast: rows 0/32/64 are ones, rest zero
            ones3 = constp.tile([96, 128], BF16)
            nc.vector.memset(ones3[:], 0.0)
            for j in range(3):
                nc.vector.memset(ones3[32 * j : 32 * j + 1, :], 1.0)

            for img in range(BL):
                # padded input planes (bf16), data at rows/cols 1..56
                xpad = xpadp.tile([128, KCH, HP, HP], BF16)
                # zero the pad strips (rows 0,57 and cols 0,57)
                nc.vector.memset(xpad[:, :, 0, :], 0.0)
                nc.vector.memset(xpad[:, :, HP - 1, :], 0.0)
                nc.vector.memset(xpad[:, :, :, 0], 0.0)
                nc.vector.memset(xpad[:, :, :, HP - 1], 0.0)

                for rg in range(RG):
                    r0 = 1 + rg * RROWS
                    for k in range(KCH):
                        xst = xstage.tile([128, NT], F32)
                        nc.sync.dma_start(
                            xst[:], xin_d[img, k, :, rg * NT : (rg + 1) * NT]
                        )
                        nc.vector.tensor_copy(
                            xpad[:, k, r0 : r0 + RROWS, 1 : 1 + W],
                            xst.rearrange("p (r w) -> p r w", r=RROWS),
                        )

                # H3: relu(h) at 3 dx-shifted partition groups (bases 0/32/64)
                h3 = h3p.tile([96, HP, HP], BF16)
                nc.vector.memset(h3[:], 0.0)

                for rg in range(RG):
                    r0 = 1 + rg * RROWS
                    hps = ps_h.tile([16, NT], F32)
                    n = 0
                    for k in range(KCH):
                        for t in range(9):
                            dy, dx = t // 3 - 1, t % 3 - 1
                            nc.tensor.matmul(
                                hps[:],
                                w1sb[:, k, t, :],
                                xpad[
                                    :, k,
                                    r0 + dy : r0 + dy + RROWS,
                                    1 + dx : 1 + dx + W,
                                ],
                                start=(n == 0),
                                stop=(n == 17),
                            )
                            n += 1
                    # h3[(32j+c), r, cc] = relu(h)(c, r, cc + j - 1)
                    for j in range(3):
                        nc.scalar.activation(
                            h3[32 * j : 32 * j + 16, r0 : r0 + RROWS, 2 - j : 2 - j + W],
                            hps.rearrange("p (r w) -> p r w", r=RROWS),
                            mybir.ActivationFunctionType.Relu,
                        )

                # A3: sigmoid(conv2) at 3 dx-shifted partitions (bases 0/32/64)
                a3 = a3p.tile([96, HP, HP], BF16)
                nc.vector.memset(a3[:], 0.0)

                for rg in range(RG):
                    r0 = 1 + rg * RROWS
                    aps = ps_a.tile([1, NT], F32)
                    for d in range(3):
                        dy = d - 1
                        nc.tensor.matmul(
                            aps[:],
                            w2sb[:, d : d + 1],
                            h3[:, r0 + dy : r0 + dy + RROWS, 1 : 1 + W],
                            start=(d == 0),
                            stop=(d == 2),
                        )
                    nc.scalar.activation(
                        a3[32:33, r0 : r0 + RROWS, 1 : 1 + W],
                        aps.rearrange("p (r w) -> p r w", r=RROWS),
                        mybir.ActivationFunctionType.Sigmoid,
                    )

                # shifted copies: a3[0,r,c] = A(r,c-1), a3[64,r,c] = A(r,c+1)
                nc.sync.dma_start(
                    a3[0:1, 1 : 1 + H, 1 : 1 + W], a3[32:33, 1 : 1 + H, 0:W]
                )
                nc.sync.dma_start(
                    a3[64:65, 1 : 1 + H, 1 : 1 + W], a3[32:33, 1 : 1 + H, 2 : 2 + W]
                )

                # box3x3 + broadcast to 128 partitions, then multiply with x_out
                for rg in range(RG):
                    r0 = 1 + rg * RROWS
                    bps = ps_b.tile([128, NT], F32)
                    for d in range(3):
                        dy = d - 1
                        nc.tensor.matmul(
                            bps[:],
                            ones3[:],
                            a3[:, r0 + dy : r0 + dy + RROWS, 1 : 1 + W],
                            start=(d == 0),
                            stop=(d == 2),
                        )
                    for k in range(KCH):
                        xo = xop.tile([128, NT], F32)
                        nc.sync.dma_start(
                            xo[:], xout_d[img, k, :, rg * NT : (rg + 1) * NT]
                        )
                        ot = otp.tile([128, NT], F32)
                        nc.vector.tensor_mul(ot[:], xo[:], bps[:])
                        nc.sync.dma_start(
                            out_d[img, k, :, rg * NT : (rg + 1) * NT], ot[:]
                        )

    nc.compile()
    return nc


def _prep_shards(x_in, x_out, w1, w2):
    bf16 = ml_dtypes.bfloat16
    w1t = np.ascontiguousarray(
        w1.reshape(16, KCH, 128, 9).transpose(1, 3, 2, 0)
    ).astype(bf16)
    w2t = np.zeros((96, 3), dtype=bf16)
    # w2t[32*j + c, d] = w2[0, c, d, j]   (j = dx index, d = dy index)
    for j in range(3):
        w2t[32 * j : 32 * j + 16, :] = w2[0, :, :, j].astype(bf16)
    xi = x_in.reshape(NCORES, BL, KCH, 128, HW)
    xo = x_out.reshape(NCORES, BL, KCH, 128, HW)
    return [
        {
            "xin": np.ascontiguousarray(xi[i]).astype(np.float32),
            "xout": np.ascontiguousarray(xo[i]).astype(np.float32),
            "w1t": w1t,
            "w2t": w2t,
        }
        for i in range(NCORES)
    ]


def _run(in_maps, trace=False):
    if "nc" not in _cache:
        _cache["nc"] = _build()
    return run_bass_kernel_spmd(
        _cache["nc"], in_maps, core_ids=list(range(NCORES)), trace=trace
    )


def kernel(x_in, x_out, w1, w2, _trace=False):
    in_maps = _prep_shards(
        np.asarray(x_in, dtype=np.float32),
        np.asarray(x_out, dtype=np.float32),
        np.asarray(w1, dtype=np.float32),
        np.asarray(w2, dtype=np.float32),
    )
    res = _run(in_maps, trace=_trace)
    out = np.stack([res.results[i]["out"] for i in range(NCORES)])
    kernel.last_exec_time_ns = res.exec_time_ns
    return out.reshape(B, C, H, W).astype(np.float32)


# revision 10
# speedup vs baseline: 88.7459x; 88.7459x over previous
"""ALayer kernel for 8 TRN2 NeuronCores — pure data parallel over batch.

Per-core shard: 4 images of [256, 56, 56].
  h  = relu(conv3x3(x_in, w1))      # 256 -> 16 ch
  A  = sigmoid(conv3x3(h, w2))      # 16 -> 1 ch
  out = x_out * box3x3(A)           # broadcast over 256 ch

TensorEngine formulation (bf16 matmuls, fp32 PSUM accumulation):
  conv1: 18 accumulating shift-matmuls (2 K-chunks of 128 in-ch x 9 taps,
         M=16 out-ch) over zero-padded 58x58 planes, 14 output rows per tile.
  conv2: relu(h) is stored at 3 dx-shifted partition groups (H3, bases
         0/32/64), so conv2 is 3 accumulating K=96 matmuls (one per dy).
  box+broadcast: sigmoid output A is stored at 3 dx-shifted partition bases
         (A3); 3 accumulating matmuls with a rows-0/32/64-ones lhsT produce
         box3x3(A) replicated to 128 partitions directly in PSUM.
  final: DVE multiply of x_out by the broadcast PSUM tile.
"""

import numpy as np
import ml_dtypes

import concourse.bass as bass
import concourse.tile as tile
import concourse.mybir as mybir
from concourse import bacc
from concourse.bass_utils import run_bass_kernel_spmd

BF16 = mybir.dt.bfloat16
F32 = mybir.dt.float32

B, C, H, W = 32, 256, 56, 56
NCORES = 8
BL = B // NCORES          # images per core
KCH = 2                   # 256 = 2 chunks of 128
HP = H + 2                # padded plane side
HW = H * W                # 3136
RG = 7                    # row groups per image
RROWS = H // RG           # 8 rows per group
NT = RROWS * W            # 448 px per tile

_cache = {}


def _build():
    nc = bacc.Bacc("TRN2", target_bir_lowering=False, debug=False)

    xin_d = nc.dram_tensor("xin", [BL, KCH, 128, HW], F32, kind="ExternalInput").ap()
    xout_d = nc.dram_tensor("xout", [BL, KCH, 128, HW], F32, kind="ExternalInput").ap()
    w1_d = nc.dram_tensor("w1t", [KCH, 9, 128, 16], BF16, kind="ExternalInput").ap()
    w2_d = nc.dram_tensor("w2t", [96, 3], BF16, kind="ExternalInput").ap()
    out_d = nc.dram_tensor("out", [BL, KCH, 128, HW], F32, kind="ExternalOutput").ap()

    with tile.TileContext(nc) as tc:
        with (
            tc.tile_pool(name="const", bufs=1) as constp,
            tc.tile_pool(name="xstage", bufs=2) as xstage,
            tc.tile_pool(name="xpad", bufs=2) as xpadp,
            tc.tile_pool(name="h3", bufs=2) as h3p,
            tc.tile_pool(name="a3", bufs=2) as a3p,
            tc.tile_pool(name="xo", bufs=2) as xop,
            tc.tile_pool(name="ot", bufs=2) as otp,
            tc.tile_pool(name="ps_ha", bufs=3, space="PSUM") as ps_ha,
            tc.tile_pool(name="ps_b", bufs=3, space="PSUM") as ps_b,
        ):
            # weights
            w1sb = constp.tile([128, KCH, 9, 16], BF16)
            nc.sync.dma_start(w1sb[:], w1_d.transpose([2, 0, 1, 3]))
            w2sb = constp.tile([96, 3], BF16)
            nc.sync.dma_start(w2sb[:], w2_d[:])
            # lhsT for fused box+broadcast: rows 0/32/64 ones, rest zero
            ones3 = constp.tile([96, 128], BF16)
            nc.vector.memset(ones3[:], 0.0)
            for j in range(3):
                nc.vector.memset(ones3[32 * j : 32 * j + 1, :], 1.0)

            for img in range(BL):
                # ---- load + cast x_in into padded bf16 planes ----
                xpad = xpadp.tile([128, KCH, HP, HP], BF16)
                if img < 2:  # first use of each pool slot: zero the pads
                    nc.gpsimd.memset(xpad[:], 0.0)

                xst = xstage.tile([128, KCH, HW], F32)
                for k in range(KCH):
                    nc.sync.dma_start(xst[:, k, :], xin_d[img, k, :, :])
                    for hhalf in range(2):
                        r0 = 1 + hhalf * (H // 2)
                        nc.gpsimd.tensor_copy(
                            xpad[:, k, r0 : r0 + H // 2, 1 : 1 + W],
                            xst[:, k, hhalf * (HW // 2) : (hhalf + 1) * (HW // 2)]
                            .rearrange("p (r w) -> p r w", r=H // 2),
                        )

                # ---- conv1 (+relu) -> H3 ----
                h3 = h3p.tile([96, HP, HP], BF16)
                if img < 2:
                    nc.gpsimd.memset(h3[:], 0.0)

                for rg in range(RG):
                    r0 = 1 + rg * RROWS
                    hps = ps_ha.tile([16, NT], F32, tag="ps")
                    n = 0
                    for k in range(KCH):
                        for t in range(9):
                            dy, dx = t // 3 - 1, t % 3 - 1
                            nc.tensor.matmul(
                                hps[:],
                                w1sb[:, k, t, :],
                                xpad[
                                    :, k,
                                    r0 + dy : r0 + dy + RROWS,
                                    1 + dx : 1 + dx + W,
                                ],
                                start=(n == 0),
                                stop=(n == 17),
                            )
                            n += 1
                    # center copy (j=1) via ACT relu; j=0 / j=2 via DVE copies
                    nc.scalar.activation(
                        h3[32:48, r0 : r0 + RROWS, 1 : 1 + W],
                        hps.rearrange("p (r w) -> p r w", r=RROWS),
                        mybir.ActivationFunctionType.Relu,
                    )
                for rg in range(RG):
                    r0 = 1 + rg * RROWS
                    nc.vector.tensor_copy(
                        h3[0:16, r0 : r0 + RROWS, 1:HP],
                        h3[32:48, r0 : r0 + RROWS, 0 : HP - 1],
                    )
                    nc.vector.tensor_copy(
                        h3[64:80, r0 : r0 + RROWS, 0 : HP - 1],
                        h3[32:48, r0 : r0 + RROWS, 1:HP],
                    )

                # ---- conv2 + sigmoid -> A3 ----
                a3 = a3p.tile([96, HP, HP], BF16)
                if img < 2:
                    nc.gpsimd.memset(a3[:], 0.0)

                for rg in range(RG):
                    r0 = 1 + rg * RROWS
                    aps = ps_ha.tile([1, NT], F32, tag="ps")
                    for d in range(3):
                        dy = d - 1
                        nc.tensor.matmul(
                            aps[:],
                            w2sb[:, d : d + 1],
                            h3[:, r0 + dy : r0 + dy + RROWS, 1 : 1 + W],
                            start=(d == 0),
                            stop=(d == 2),
                        )
                    nc.scalar.activation(
                        a3[32:33, r0 : r0 + RROWS, 1 : 1 + W],
                        aps.rearrange("p (r w) -> p r w", r=RROWS),
                        mybir.ActivationFunctionType.Sigmoid,
                    )

                # shifted copies: a3[0,r,c] = A(r,c-1), a3[64,r,c] = A(r,c+1)
                nc.sync.dma_start(
                    a3[0:1, 1 : 1 + H, 1 : 1 + W], a3[32:33, 1 : 1 + H, 0:W]
                )
                nc.sync.dma_start(
                    a3[64:65, 1 : 1 + H, 1 : 1 + W], a3[32:33, 1 : 1 + H, 2 : 2 + W]
                )

                # ---- box3x3 + broadcast, multiply with x_out, store ----
                xo = xop.tile([128, KCH, HW], F32)
                for k in range(KCH):
                    nc.sync.dma_start(xo[:, k, :], xout_d[img, k, :, :])
                ot = otp.tile([128, KCH, HW], F32)

                for rg in range(RG):
                    r0 = 1 + rg * RROWS
                    bps = ps_b.tile([128, NT], F32)
                    for d in range(3):
                        dy = d - 1
                        nc.tensor.matmul(
                            bps[:],
                            ones3[:],
                            a3[:, r0 + dy : r0 + dy + RROWS, 1 : 1 + W],
                            start=(d == 0),
                            stop=(d == 2),
                        )
                    for k in range(KCH):
                        nc.vector.tensor_mul(
                            ot[:, k, rg * NT : (rg + 1) * NT],
                            xo[:, k, rg * NT : (rg + 1) * NT],
                            bps[:],
                        )
                for k in range(KCH):
                    nc.scalar.dma_start(out_d[img, k, :, :], ot[:, k, :])

    nc.compile()
    return nc


def _prep_shards(x_in, x_out, w1, w2):
    bf16 = ml_dtypes.bfloat16
    w1t = np.ascontiguousarray(
        w1.reshape(16, KCH, 128, 9).transpose(1, 3, 2, 0)
    ).astype(bf16)
    w2t = np.zeros((96, 3), dtype=bf16)
    # w2t[32*j + c, d] = w2[0, c, d, j]   (j = dx index, d = dy index)
    for j in range(3):
        w2t[32 * j : 32 * j + 16, :] = w2[0, :, :, j].astype(bf16)
    xi = x_in.reshape(NCORES, BL, KCH, 128, HW)
    xo = x_out.reshape(NCORES, BL, KCH, 128, HW)
    return [
        {
            "xin": np.ascontiguousarray(xi[i]).astype(np.float32),
            "xout": np.ascontiguousarray(xo[i]).astype(np.float32),
            "w1t": w1t,
            "w2t": w2t,
        }
        for i in range(NCORES)
    ]


def _run(in_maps, trace=False):
    if "nc" not in _cache:
        _cache["nc"] = _build()
    return run_bass_kernel_spmd(
        _cache["nc"], in_maps, core_ids=list(range(NCORES)), trace=trace
    )


def kernel(x_in, x_out, w1, w2, _trace=False):
    in_maps = _prep_shards(
        np.asarray(x_in, dtype=np.float32),
        np.asarray(x_out, dtype=np.float32),
        np.asarray(w1, dtype=np.float32),
        np.asarray(w2, dtype=np.float32),
    )
    res = _run(in_maps, trace=_trace)
    out = np.stack([res.results[i]["out"] for i in range(NCORES)])
    kernel.last_exec_time_ns = res.exec_time_ns
    return out.reshape(B, C, H, W).astype(np.float32)


# revision 14
# speedup vs baseline: 122.6795x; 1.3824x over previous
"""ALayer kernel for 8 TRN2 NeuronCores — pure data parallel over batch.

Per-core shard: 4 images of [256, 56, 56].
  h  = relu(conv3x3(x_in, w1))      # 256 -> 16 ch
  A  = sigmoid(conv3x3(h, w2))      # 16 -> 1 ch
  out = x_out * box3x3(A)           # broadcast over 256 ch

TensorEngine formulation (bf16 matmuls, fp32 PSUM accumulation):
  conv1: 18 accumulating shift-matmuls (2 K-chunks of 128 in-ch x 9 taps,
         M=16 out-ch) over zero-padded 58x58 planes, 14 output rows per tile.
  conv2: relu(h) is stored at 3 dx-shifted partition groups (H3, bases
         0/32/64), so conv2 is 3 accumulating K=96 matmuls (one per dy).
  box+broadcast: sigmoid output A is stored at 3 dx-shifted partition bases
         (A3); 3 accumulating matmuls with a rows-0/32/64-ones lhsT produce
         box3x3(A) replicated to 128 partitions directly in PSUM.
  final: DVE multiply of x_out by the broadcast PSUM tile.
"""

import numpy as np
import ml_dtypes

import concourse.bass as bass
import concourse.tile as tile
import concourse.mybir as mybir
from concourse import bacc
from concourse.bass_utils import run_bass_kernel_spmd

BF16 = mybir.dt.bfloat16
F32 = mybir.dt.float32

B, C, H, W = 32, 256, 56, 56
NCORES = 8
BL = B // NCORES          # images per core
KCH = 2                   # 256 = 2 chunks of 128
HP = H + 2                # padded plane side
HW = H * W                # 3136
RG = 7                    # row groups per image
RROWS = H // RG           # 8 rows per group
NT = RROWS * W            # 448 px per tile

_cache = {}


def _build():
    nc = bacc.Bacc("TRN2", target_bir_lowering=False, debug=False)

    xin_d = nc.dram_tensor("xin", [BL, KCH, 128, HW], F32, kind="ExternalInput").ap()
    xout_d = nc.dram_tensor("xout", [BL, KCH, 128, HW], F32, kind="ExternalInput").ap()
    w1_d = nc.dram_tensor("w1t", [KCH, 9, 128, 16], BF16, kind="ExternalInput").ap()
    w2_d = nc.dram_tensor("w2t", [96, 3], BF16, kind="ExternalInput").ap()
    out_d = nc.dram_tensor("out", [BL, KCH, 128, HW], F32, kind="ExternalOutput").ap()

    with tile.TileContext(nc) as tc:
        with (
            tc.tile_pool(name="const", bufs=1) as constp,
            tc.tile_pool(name="xstage", bufs=2) as xstage,
            tc.tile_pool(name="xpad", bufs=2) as xpadp,
            tc.tile_pool(name="h3", bufs=2) as h3p,
            tc.tile_pool(name="a3", bufs=2) as a3p,
            tc.tile_pool(name="xo", bufs=2) as xop,
            tc.tile_pool(name="ot", bufs=2) as otp,
            tc.tile_pool(name="ps_ha", bufs=3, space="PSUM") as ps_ha,
            tc.tile_pool(name="ps_b", bufs=3, space="PSUM") as ps_b,
        ):
            # weights
            w1sb = constp.tile([128, KCH, 9, 16], BF16)
            nc.sync.dma_start(w1sb[:], w1_d.transpose([2, 0, 1, 3]))
            w2sb = constp.tile([96, 3], BF16)
            nc.sync.dma_start(w2sb[:], w2_d[:])
            # lhsT for fused box+broadcast: rows 0/32/64 ones, rest zero
            ones3 = constp.tile([96, 128], BF16)
            nc.vector.memset(ones3[:], 0.0)
            for j in range(3):
                nc.vector.memset(ones3[32 * j : 32 * j + 1, :], 1.0)

            for img in range(BL):
                # ---- load + cast x_in into padded bf16 planes ----
                xpad = xpadp.tile([128, KCH, HP, HP], BF16)
                if img < 2:  # first use of each pool slot: zero the pads
                    nc.vector.memset(xpad[:], 0.0)

                xst = xstage.tile([128, KCH, HW], F32)
                for k in range(KCH):
                    nc.sync.dma_start(xst[:, k, :], xin_d[img, k, :, :])
                    for hhalf in range(2):
                        r0 = 1 + hhalf * (H // 2)
                        nc.vector.tensor_copy(
                            xpad[:, k, r0 : r0 + H // 2, 1 : 1 + W],
                            xst[:, k, hhalf * (HW // 2) : (hhalf + 1) * (HW // 2)]
                            .rearrange("p (r w) -> p r w", r=H // 2),
                        )

                # ---- conv1 (+relu) -> H3 ----
                h3 = h3p.tile([96, HP, HP], BF16)
                if img < 2:
                    nc.vector.memset(h3[:], 0.0)

                for rg in range(RG):
                    r0 = 1 + rg * RROWS
                    hps = ps_ha.tile([16, NT], F32, tag="ps")
                    n = 0
                    for k in range(KCH):
                        for t in range(9):
                            dy, dx = t // 3 - 1, t % 3 - 1
                            nc.tensor.matmul(
                                hps[:],
                                w1sb[:, k, t, :],
                                xpad[
                                    :, k,
                                    r0 + dy : r0 + dy + RROWS,
                                    1 + dx : 1 + dx + W,
                                ],
                                start=(n == 0),
                                stop=(n == 17),
                            )
                            n += 1
                    # center copy (j=1) via ACT relu; j=0 / j=2 via DVE copies
                    nc.scalar.activation(
                        h3[32:48, r0 : r0 + RROWS, 1 : 1 + W],
                        hps.rearrange("p (r w) -> p r w", r=RROWS),
                        mybir.ActivationFunctionType.Relu,
                    )
                # flat whole-plane shifted copies (padded layout makes the
                # flat +-1 shift exactly the dx shift with correct zero pads)
                h3f = h3.rearrange("p r w -> p (r w)")
                PL = HP * HP
                nc.vector.tensor_copy(h3f[0:16, 1:PL], h3f[32:48, 0 : PL - 1])
                nc.vector.tensor_copy(h3f[64:80, 0 : PL - 1], h3f[32:48, 1:PL])

                # ---- conv2 + sigmoid -> A3 ----
                a3 = a3p.tile([96, HP, HP], BF16)
                if img < 2:
                    nc.vector.memset(a3[:], 0.0)

                for rg in range(RG):
                    r0 = 1 + rg * RROWS
                    aps = ps_ha.tile([1, NT], F32, tag="ps")
                    for d in range(3):
                        dy = d - 1
                        nc.tensor.matmul(
                            aps[:],
                            w2sb[:, d : d + 1],
                            h3[:, r0 + dy : r0 + dy + RROWS, 1 : 1 + W],
                            start=(d == 0),
                            stop=(d == 2),
                        )
                    nc.scalar.activation(
                        a3[32:33, r0 : r0 + RROWS, 1 : 1 + W],
                        aps.rearrange("p (r w) -> p r w", r=RROWS),
                        mybir.ActivationFunctionType.Sigmoid,
                    )

                # shifted copies: a3[0,r,c] = A(r,c-1), a3[64,r,c] = A(r,c+1)
                nc.sync.dma_start(
                    a3[0:1, 1 : 1 + H, 1 : 1 + W], a3[32:33, 1 : 1 + H, 0:W]
                )
                nc.sync.dma_start(
                    a3[64:65, 1 : 1 + H, 1 : 1 + W], a3[32:33, 1 : 1 + H, 2 : 2 + W]
                )

                # ---- box3x3 + broadcast, multiply with x_out, store ----
                xo = xop.tile([128, KCH, HW], F32)
                for k in range(KCH):
                    nc.sync.dma_start(xo[:, k, :], xout_d[img, k, :, :])
                ot = otp.tile([128, KCH, HW], F32)

                for rg in range(RG):
                    r0 = 1 + rg * RROWS
                    bps = ps_b.tile([128, NT], F32)
                    for d in range(3):
                        dy = d - 1
                        nc.tensor.matmul(
                            bps[:],
                            ones3[:],
                            a3[:, r0 + dy : r0 + dy + RROWS, 1 : 1 + W],
                            start=(d == 0),
                            stop=(d == 2),
                        )
                    for k in range(KCH):
                        nc.vector.tensor_mul(
                            ot[:, k, rg * NT : (rg + 1) * NT],
                            xo[:, k, rg * NT : (rg + 1) * NT],
                            bps[:],
                        )
                for k in range(KCH):
                    nc.scalar.dma_start(out_d[img, k, :, :], ot[:, k, :])

    nc.compile()
    return nc


def _prep_shards(x_in, x_out, w1, w2):
    bf16 = ml_dtypes.bfloat16
    w1t = np.ascontiguousarray(
        w1.reshape(16, KCH, 128, 9).transpose(1, 3, 2, 0)
    ).astype(bf16)
    w2t = np.zeros((96, 3), dtype=bf16)
    # w2t[32*j + c, d] = w2[0, c, d, j]   (j = dx index, d = dy index)
    for j in range(3):
        w2t[32 * j : 32 * j + 16, :] = w2[0, :, :, j].astype(bf16)
    xi = x_in.reshape(NCORES, BL, KCH, 128, HW)
    xo = x_out.reshape(NCORES, BL, KCH, 128, HW)
    return [
        {
            "xin": np.ascontiguousarray(xi[i]).astype(np.float32),
            "xout": np.ascontiguousarray(xo[i]).astype(np.float32),
            "w1t": w1t,
            "w2t": w2t,
        }
        for i in range(NCORES)
    ]


def _run(in_maps, trace=False):
    if "nc" not in _cache:
        _cache["nc"] = _build()
    return run_bass_kernel_spmd(
        _cache["nc"], in_maps, core_ids=list(range(NCORES)), trace=trace
    )


def kernel(x_in, x_out, w1, w2, _trace=False):
    in_maps = _prep_shards(
        np.asarray(x_in, dtype=np.float32),
        np.asarray(x_out, dtype=np.float32),
        np.asarray(w1, dtype=np.float32),
        np.asarray(w2, dtype=np.float32),
    )
    res = _run(in_maps, trace=_trace)
    out = np.stack([res.results[i]["out"] for i in range(NCORES)])
    kernel.last_exec_time_ns = res.exec_time_ns
    return out.reshape(B, C, H, W).astype(np.float32)
